# revision 12
# baseline (speedup 1.0000x reference)
"""Trainium2 Bass kernel for the paired-view ("flip") multi-head attention module.

Full computation (reference semantics, B=2 P=2 S=1024 D=1024 H=16):
    q/k/v = Linear(x) -> [B,P,H,S,DK]
    left  = softmax(q k^T / 8 + mask) v          (same pair index)
    right = softmax(q k_flip^T / 8 + mask) v_flip (pair index swapped)
    out   = (left + 0.1*tanh(right)) @ Wo.T + bo

Sharding over 8 NeuronCores: data-parallel on B (2 groups of 4 cores),
tensor-parallel on heads within a group (4 heads/core, 256 channels).
Each core computes its heads' projections (column-parallel), full attention
for its heads over both pair views, and a row-parallel partial of the output
projection.  The host sums the 4 partials per batch and adds bo.

Key layout trick: scores are computed TRANSPOSED ([k, q] instead of [q, k])
so softmax's exp is orientation-free and the attention-value product needs
no on-chip transposes; row sums come free via an extra ones-column in V.
All matmuls run in float32r (TF32-like, full PE rate at N>=512).
"""

import numpy as np

import concourse.bass as bass
import concourse.tile as tile
from concourse import bacc, mybir
from concourse.bass_utils import run_bass_kernel_spmd

F32 = mybir.dt.float32
F32R = mybir.dt.float32r
BF16 = mybir.dt.bfloat16
I32 = mybir.dt.int32

# per-stage matmul input dtypes (both operands of a matmul must match)
X_DT = BF16    # projection inputs: xT staging + Wq/Wk/Wv
QK_DT = BF16   # q/k tiles feeding the scores matmul
AV_DT = BF16   # exp(scores) + v_aug feeding the AV matmul
OUT_DT = BF16  # combine + Wo feeding the output projection
AF = mybir.ActivationFunctionType
OP = mybir.AluOpType

B, P, S, D, H = 2, 2, 1024, 1024, 16
DK = D // H          # 64
NCORES = 8
GROUP = 4            # cores per batch entry
NH = H // GROUP      # 4 local heads per core
CH = NH * DK         # 256 local channels
R = P * S            # 2048 rows per batch entry
KC = 8               # d_model chunks of 128
RB = 4               # row banks of 512
MASK_NEG = 60.0      # exp(-60) == 0 relative to any sum


def _emit(nc, tc, xq, xk, xv, wq, wk, wv, wo, bq, bk, bv, mask, out_d):
    from contextlib import ExitStack

    with ExitStack() as ctx:
        sb = ctx.enter_context(tc.tile_pool(name="sb", bufs=1))
        ps = ctx.enter_context(tc.tile_pool(name="ps", bufs=1, space="PSUM"))
        _body(nc, sb, ps, xq, xk, xv, wq, wk, wv, wo, bq, bk, bv, mask, out_d)


def _body(nc, sb, ps, xq, xk, xv, wq, wk, wv, wo, bq, bk, bv, mask, out_d):
    # ---- constants ----------------------------------------------------
    wq_sb = sb.tile([128, KC * CH], X_DT, name="wq_sb")
    wk_sb = sb.tile([128, KC * CH], X_DT, name="wk_sb")
    wv_sb = sb.tile([128, KC * CH], X_DT, name="wv_sb")
    for t_d, t_s in ((wq, wq_sb), (wk, wk_sb), (wv, wv_sb)):
        nc.sync.dma_start(
            out=t_s[:].rearrange("p (kc c) -> p kc c", kc=KC),
            in_=t_d[:].rearrange("(kc p) c -> p kc c", p=128),
        )
    wo_sb = sb.tile([128, 2 * D], OUT_DT, name="wo_sb")
    nc.sync.dma_start(
        out=wo_sb[:].rearrange("p (kk c) -> p kk c", kk=2),
        in_=wo[:].rearrange("(kk p) c -> p kk c", p=128),
    )

    bq_sb = sb.tile([128, 2], F32, name="bq_sb")
    bk_sb = sb.tile([128, 2], F32, name="bk_sb")
    nc.sync.dma_start(out=bq_sb[:], in_=bq[:].rearrange("(mo p) -> p mo", p=128))
    nc.sync.dma_start(out=bk_sb[:], in_=bk[:].rearrange("(mo p) -> p mo", p=128))
    bv_row = sb.tile([1, CH], F32, name="bv_row")
    nc.sync.dma_start(out=bv_row[:], in_=bv[None, :])
    bv_bc = sb.tile([128, CH], F32, name="bv_bc")
    nc.gpsimd.partition_broadcast(bv_bc[:], bv_row[:])

    # mask as a per-row 0/1 multiplier on v_aug (kills masked keys in both
    # the attention numerator and the ones-column denominator)
    mask_sb = sb.tile([128, 2 * KC], I32, name="mask_sb")
    nc.sync.dma_start(
        out=mask_sb[:],
        in_=mask[:].rearrange("pp (kc p) -> p pp kc", p=128),
    )
    maskf = sb.tile([128, 2 * KC], F32, name="maskf")
    nc.vector.tensor_scalar(
        out=maskf[:], in0=mask_sb[:], scalar1=1.0, scalar2=None, op0=OP.mult,
    )

    # ---- projections --------------------------------------------------
    # qT/kT: [o_local, p*S + s] in 2 tiles of 128 channels (2 heads each)
    qT = [sb.tile([128, R], QK_DT, name=f"qT{mo}") for mo in range(2)]
    kT = [sb.tile([128, R], QK_DT, name=f"kT{mo}") for mo in range(2)]
    # v_aug: [r_local, rc(16) x (h(4) x 65)]; col h*65+64 holds ones
    v_aug = sb.tile([128, 16 * NH * 65], AV_DT, name="v_aug")
    nc.gpsimd.memset(v_aug[:], 1.0)

    for src_d, kind in ((xk, "k"), (xv, "v"), (xq, "q")):
        w_sb = {"q": wq_sb, "k": wk_sb, "v": wv_sb}[kind]
        for rb in range(RB):
            stage = sb.tile([128, KC * 512], X_DT, name="stage", tag="stage", bufs=3)
            nc.sync.dma_start(
                out=stage[:].rearrange("p (kc c) -> p kc c", kc=KC),
                in_=src_d[:, rb * 512 : (rb + 1) * 512].rearrange(
                    "(kc p) c -> p kc c", p=128
                ),
            )
            if kind in ("q", "k"):
                dst, b_sb = (qT, bq_sb) if kind == "q" else (kT, bk_sb)
                for mo in range(2):
                    pp_t = ps.tile([128, 512], F32, name="ps_proj", tag="ps_proj", bufs=2)
                    for kc in range(KC):
                        nc.tensor.matmul(
                            pp_t[:],
                            w_sb[:, kc * CH + mo * 128 : kc * CH + (mo + 1) * 128],
                            stage[:, kc * 512 : (kc + 1) * 512],
                            start=(kc == 0),
                            stop=(kc == KC - 1),
                        )
                    nc.vector.tensor_scalar(
                        out=dst[mo][:, rb * 512 : (rb + 1) * 512],
                        in0=pp_t[:],
                        scalar1=b_sb[:, mo : mo + 1],
                        scalar2=None,
                        op0=OP.add,
                    )
            else:
                for rs in range(4):
                    rc = rb * 4 + rs
                    pv_t = ps.tile([128, CH], F32, name="ps_v", tag="ps_proj", bufs=2)
                    for kc in range(KC):
                        nc.tensor.matmul(
                            pv_t[:],
                            stage[:, kc * 512 + rs * 128 : kc * 512 + (rs + 1) * 128],
                            wv_sb[:, kc * CH : (kc + 1) * CH],
                            start=(kc == 0),
                            stop=(kc == KC - 1),
                        )
                    # (v + bias) packed into the strided 65-wide layout, then
                    # scaled by the per-row mask (column rc = pp*KC+kc index)
                    vtmp = sb.tile([128, CH], F32, name="vtmp", tag="vtmp", bufs=2)
                    nc.vector.tensor_tensor(
                        out=vtmp[:], in0=pv_t[:], in1=bv_bc[:], op=OP.add
                    )
                    dst_ap = v_aug[
                        :, rc * NH * 65 : (rc + 1) * NH * 65
                    ].rearrange("p (h x) -> p h x", h=NH)[:, :, 0:DK]
                    nc.vector.tensor_scalar(
                        out=dst_ap,
                        in0=vtmp[:].rearrange("p (h x) -> p h x", h=NH),
                        scalar1=maskf[:, rc : rc + 1],
                        scalar2=None,
                        op0=OP.mult,
                    )
                    # mask the ones column too
                    nc.vector.tensor_scalar(
                        out=v_aug[
                            :, rc * NH * 65 : (rc + 1) * NH * 65
                        ].rearrange("p (h x) -> p h x", h=NH)[:, :, DK : DK + 1],
                        in0=maskf[:, rc : rc + 1]
                        .rearrange("p (a b) -> p a b", a=1)
                        .broadcast_to([128, NH, 1]),
                        scalar1=1.0,
                        scalar2=None,
                        op0=OP.mult,
                    )

    # ---- attention + per-p combine + output projection ----------------
    comb = [sb.tile([128, R], OUT_DT, name=f"comb{kk}") for kk in range(2)]
    for p in range(P):
        avs = []
        for h in range(NH):
            for side in range(2):
                pp = p if side == 0 else 1 - p
                mo, po = h // 2, (h % 2) * 64
                ex = [
                    sb.tile([128, 4096], AV_DT, name="ex", tag="stage", bufs=3)
                    for _ in range(2)
                ]
                for kc in range(KC):
                    ss_t = ps.tile([128, 1024], F32, name="ps_s", tag="ps_s", bufs=3)
                    for qb in range(2):
                        nc.tensor.matmul(
                            ss_t[:, qb * 512 : (qb + 1) * 512],
                            kT[mo][po : po + 64, pp * S + kc * 128 : pp * S + (kc + 1) * 128],
                            qT[mo][po : po + 64, p * S + qb * 512 : p * S + (qb + 1) * 512],
                            start=True,
                            stop=True,
                        )
                    nc.scalar.activation(
                        ex[kc // 4][:, (kc % 4) * 1024 : (kc % 4 + 1) * 1024],
                        ss_t[:],
                        AF.Exp,
                        scale=0.125,
                    )
                av = sb.tile([65, S], F32, name="av", tag="avT", bufs=8)
                for qb in range(2):
                    pa_t = ps.tile([65, 512], F32, name="ps_av", tag="ps_proj", bufs=2)
                    for kc in range(KC):
                        nc.tensor.matmul(
                            pa_t[:],
                            v_aug[:, (pp * KC + kc) * NH * 65 + h * 65 : (pp * KC + kc) * NH * 65 + (h + 1) * 65],
                            ex[kc // 4][:, (kc % 4) * 1024 + qb * 512 : (kc % 4) * 1024 + (qb + 1) * 512],
                            start=(kc == 0),
                            stop=(kc == KC - 1),
                        )
                    nc.vector.tensor_copy(av[:, qb * 512 : (qb + 1) * 512], pa_t[:])
                avs.append(av)

        # normalization: fold the 8 sums rows to [128, 64] (16 partitions per
        # combo) for a cheap batched reciprocal, then unfold per combo
        sums_rs = sb.tile([128, 64], F32, name="sums_rs", tag="sums_rs", bufs=1)
        for i, av in enumerate(avs):
            nc.sync.dma_start(
                out=sums_rs[i * 16 : (i + 1) * 16, :],
                in_=av[64:65, :].rearrange("p (m e) -> p m e", e=64),
            )
        recip_rs = sb.tile([128, 64], F32, name="recip_rs", tag="recip_rs", bufs=1)
        nc.vector.reciprocal(recip_rs[:], sums_rs[:])

        for h in range(NH):
            avL, avR = avs[2 * h], avs[2 * h + 1]
            rrL = sb.tile([1, S], F32, name="rrL", tag="rrow", bufs=2)
            rrR = sb.tile([1, S], F32, name="rrR", tag="rrow", bufs=2)
            nc.sync.dma_start(
                out=rrL[:].rearrange("p (m e) -> p m e", e=64),
                in_=recip_rs[(2 * h) * 16 : (2 * h + 1) * 16, :],
            )
            nc.sync.dma_start(
                out=rrR[:].rearrange("p (m e) -> p m e", e=64),
                in_=recip_rs[(2 * h + 1) * 16 : (2 * h + 2) * 16, :],
            )
            po = (h % 2) * 64
            bcL = sb.tile([64, S], F32, name="bcL", tag="bc", bufs=2)
            bcR = sb.tile([64, S], F32, name="bcR", tag="bc", bufs=2)
            nc.gpsimd.partition_broadcast(bcL[:], rrL[:])
            nc.gpsimd.partition_broadcast(bcR[:], rrR[:])
            t1 = sb.tile([64, S], F32, name="t1", tag="t1", bufs=2)
            t2 = sb.tile([64, S], F32, name="t2", tag="t2", bufs=2)
            t3 = sb.tile([64, S], F32, name="t3", tag="t3", bufs=2)
            nc.vector.tensor_tensor(out=t1[:], in0=avL[0:64, :], in1=bcL[:], op=OP.mult)
            nc.vector.tensor_tensor(out=t2[:], in0=avR[0:64, :], in1=bcR[:], op=OP.mult)
            nc.scalar.activation(t3[:], t2[:], AF.Tanh)
            nc.vector.scalar_tensor_tensor(
                out=comb[h // 2][po : po + 64, p * S : (p + 1) * S],
                in0=t3[:],
                scalar=0.1,
                in1=t1[:],
                op0=OP.mult,
                op1=OP.add,
            )

        # ---- output projection for this p (row-parallel partial) ------
        for rc in range(8):
            od = sb.tile([128, D], F32, name="od", tag="od", bufs=2)
            for ob in range(2):
                po_t = ps.tile([128, 512], F32, name="ps_o", tag="ps_proj", bufs=2)
                for kk in range(2):
                    nc.tensor.matmul(
                        po_t[:],
                        comb[kk][:, p * S + rc * 128 : p * S + (rc + 1) * 128],
                        wo_sb[:, kk * D + ob * 512 : kk * D + (ob + 1) * 512],
                        start=(kk == 0),
                        stop=(kk == 1),
                    )
                nc.vector.tensor_copy(od[:, ob * 512 : (ob + 1) * 512], po_t[:])
            nc.sync.dma_start(
                out=out_d[p * S + rc * 128 : p * S + (rc + 1) * 128, :], in_=od[:]
            )


_CACHED = None


def _build():
    global _CACHED
    if _CACHED is not None:
        return _CACHED
    nc = bacc.Bacc("TRN2", target_bir_lowering=False, debug=False)
    xq = nc.dram_tensor("xq", [D, R], X_DT, kind="ExternalInput")
    xk = nc.dram_tensor("xk", [D, R], X_DT, kind="ExternalInput")
    xv = nc.dram_tensor("xv", [D, R], X_DT, kind="ExternalInput")
    wq = nc.dram_tensor("wq", [D, CH], X_DT, kind="ExternalInput")
    wk = nc.dram_tensor("wk", [D, CH], X_DT, kind="ExternalInput")
    wv = nc.dram_tensor("wv", [D, CH], X_DT, kind="ExternalInput")
    wo = nc.dram_tensor("wo", [CH, D], OUT_DT, kind="ExternalInput")
    bq = nc.dram_tensor("bq", [CH], F32, kind="ExternalInput")
    bk = nc.dram_tensor("bk", [CH], F32, kind="ExternalInput")
    bv = nc.dram_tensor("bv", [CH], F32, kind="ExternalInput")
    mask = nc.dram_tensor("mask", [P, S], I32, kind="ExternalInput")
    out_d = nc.dram_tensor("out", [R, D], F32, kind="ExternalOutput")
    with tile.TileContext(nc) as tc:
        _emit(nc, tc, xq, xk, xv, wq, wk, wv, wo, bq, bk, bv, mask, out_d)
    nc.compile()
    _CACHED = nc
    return nc


def _in_maps(query, key, value, mask, Wq, bq, Wk, bk, Wv, bv, Wo):
    xnp = mybir.dt.np(X_DT)
    onp = mybir.dt.np(OUT_DT)
    f32 = lambda a: np.ascontiguousarray(np.asarray(a, dtype=np.float32))
    xdt = lambda a: np.ascontiguousarray(np.asarray(a).astype(xnp))
    odt = lambda a: np.ascontiguousarray(np.asarray(a).astype(onp))
    query, key, value = f32(query), f32(key), f32(value)
    Wq, Wk, Wv, Wo = f32(Wq), f32(Wk), f32(Wv), f32(Wo)
    bq, bk, bv = f32(bq), f32(bk), f32(bv)
    mask = np.ascontiguousarray(np.asarray(mask, dtype=np.int32))

    xqT = [xdt(query[b].reshape(R, D).T) for b in range(B)]
    xkT = [xdt(key[b].reshape(R, D).T) for b in range(B)]
    xvT = [xdt(value[b].reshape(R, D).T) for b in range(B)]

    maps = []
    for c in range(NCORES):
        b, hg = divmod(c, GROUP)
        ch = slice(hg * CH, (hg + 1) * CH)
        maps.append(
            {
                "xq": xqT[b],
                "xk": xkT[b],
                "xv": xvT[b],
                "wq": xdt(Wq[ch, :].T),
                "wk": xdt(Wk[ch, :].T),
                "wv": xdt(Wv[ch, :].T),
                "wo": odt(Wo[:, ch].T),
                "bq": bq[ch],
                "bk": bk[ch],
                "bv": bv[ch],
                "mask": mask[b, :, 0, :],
            }
        )
    return maps


def _run(in_maps, **kwargs):
    nc = _build()
    return run_bass_kernel_spmd(nc, in_maps, core_ids=list(range(NCORES)), **kwargs)


def kernel(query, key, value, mask, Wq, bq, Wk, bk, Wv, bv, Wo, bo):
    res = _run(_in_maps(query, key, value, mask, Wq, bq, Wk, bk, Wv, bv, Wo))
    bo = np.asarray(bo, dtype=np.float32)
    out = np.zeros((B, P, S, D), dtype=np.float32)
    for c in range(NCORES):
        b = c // GROUP
        out[b] += res.results[c]["out"].reshape(P, S, D)
    out += bo
    return out


# revision 13
# speedup vs baseline: 1.0306x; 1.0306x over previous
"""Trainium2 Bass kernel for the paired-view ("flip") multi-head attention module.

Full computation (reference semantics, B=2 P=2 S=1024 D=1024 H=16):
    q/k/v = Linear(x) -> [B,P,H,S,DK]
    left  = softmax(q k^T / 8 + mask) v          (same pair index)
    right = softmax(q k_flip^T / 8 + mask) v_flip (pair index swapped)
    out   = (left + 0.1*tanh(right)) @ Wo.T + bo

Sharding over 8 NeuronCores: data-parallel on B (2 groups of 4 cores),
tensor-parallel on heads within a group (4 heads/core, 256 channels).
Each core computes its heads' projections (column-parallel), full attention
for its heads over both pair views, and a row-parallel partial of the output
projection.  The host sums the 4 partials per batch and adds bo.

Key layout trick: scores are computed TRANSPOSED ([k, q] instead of [q, k])
so softmax's exp is orientation-free and the attention-value product needs
no on-chip transposes; row sums come free via an extra ones-column in V.
All matmuls run in float32r (TF32-like, full PE rate at N>=512).
"""

import numpy as np

import concourse.bass as bass
import concourse.tile as tile
from concourse import bacc, mybir
from concourse.bass_utils import run_bass_kernel_spmd

F32 = mybir.dt.float32
F32R = mybir.dt.float32r
BF16 = mybir.dt.bfloat16
I32 = mybir.dt.int32

# per-stage matmul input dtypes (both operands of a matmul must match)
X_DT = BF16    # projection inputs: xT staging + Wq/Wk/Wv
QK_DT = BF16   # q/k tiles feeding the scores matmul
AV_DT = BF16   # exp(scores) + v_aug feeding the AV matmul
OUT_DT = BF16  # combine + Wo feeding the output projection
AF = mybir.ActivationFunctionType
OP = mybir.AluOpType

B, P, S, D, H = 2, 2, 1024, 1024, 16
DK = D // H          # 64
NCORES = 8
GROUP = 4            # cores per batch entry
NH = H // GROUP      # 4 local heads per core
CH = NH * DK         # 256 local channels
R = P * S            # 2048 rows per batch entry
KC = 8               # d_model chunks of 128
RB = 4               # row banks of 512
MASK_NEG = 60.0      # exp(-60) == 0 relative to any sum


def _emit(nc, tc, xq, xk, xv, wq, wk, wv, wo, bq, bk, bv, mask, out_d):
    from contextlib import ExitStack

    with ExitStack() as ctx:
        sb = ctx.enter_context(tc.tile_pool(name="sb", bufs=1))
        ps = ctx.enter_context(tc.tile_pool(name="ps", bufs=1, space="PSUM"))
        _body(nc, sb, ps, xq, xk, xv, wq, wk, wv, wo, bq, bk, bv, mask, out_d)


def _body(nc, sb, ps, xq, xk, xv, wq, wk, wv, wo, bq, bk, bv, mask, out_d):
    # ---- constants ----------------------------------------------------
    wq_sb = sb.tile([128, KC * CH], X_DT, name="wq_sb")
    wk_sb = sb.tile([128, KC * CH], X_DT, name="wk_sb")
    wv_sb = sb.tile([128, KC * CH], X_DT, name="wv_sb")
    for t_d, t_s in ((wq, wq_sb), (wk, wk_sb), (wv, wv_sb)):
        nc.gpsimd.dma_start(
            out=t_s[:].rearrange("p (kc c) -> p kc c", kc=KC),
            in_=t_d[:].rearrange("(kc p) c -> p kc c", p=128),
        )
    wo_sb = sb.tile([128, 2 * D], OUT_DT, name="wo_sb")
    nc.gpsimd.dma_start(
        out=wo_sb[:].rearrange("p (kk c) -> p kk c", kk=2),
        in_=wo[:].rearrange("(kk p) c -> p kk c", p=128),
    )

    bq_sb = sb.tile([128, 2], F32, name="bq_sb")
    bk_sb = sb.tile([128, 2], F32, name="bk_sb")
    nc.sync.dma_start(out=bq_sb[:], in_=bq[:].rearrange("(mo p) -> p mo", p=128))
    nc.sync.dma_start(out=bk_sb[:], in_=bk[:].rearrange("(mo p) -> p mo", p=128))
    bv_row = sb.tile([1, CH], F32, name="bv_row")
    nc.sync.dma_start(out=bv_row[:], in_=bv[None, :])
    bv_bc = sb.tile([128, CH], F32, name="bv_bc")
    nc.gpsimd.partition_broadcast(bv_bc[:], bv_row[:])

    # mask as a per-row 0/1 multiplier on v_aug (kills masked keys in both
    # the attention numerator and the ones-column denominator)
    mask_sb = sb.tile([128, 2 * KC], I32, name="mask_sb")
    nc.sync.dma_start(
        out=mask_sb[:],
        in_=mask[:].rearrange("pp (kc p) -> p pp kc", p=128),
    )
    maskf = sb.tile([128, 2 * KC], F32, name="maskf")
    nc.vector.tensor_scalar(
        out=maskf[:], in0=mask_sb[:], scalar1=1.0, scalar2=None, op0=OP.mult,
    )

    # ---- projections --------------------------------------------------
    # qT/kT: [o_local, p*S + s] in 2 tiles of 128 channels (2 heads each)
    qT = [sb.tile([128, R], QK_DT, name=f"qT{mo}") for mo in range(2)]
    kT = [sb.tile([128, R], QK_DT, name=f"kT{mo}") for mo in range(2)]
    # v_aug: [r_local, rc(16) x (h(4) x 65)]; col h*65+64 holds ones
    v_aug = sb.tile([128, 16 * NH * 65], AV_DT, name="v_aug")
    nc.gpsimd.memset(v_aug[:], 1.0)

    for src_d, kind in ((xk, "k"), (xv, "v"), (xq, "q")):
        w_sb = {"q": wq_sb, "k": wk_sb, "v": wv_sb}[kind]
        for rb in range(RB):
            stage = sb.tile([128, KC * 512], X_DT, name="stage", tag="stage", bufs=3)
            for half in range(2):
                eng = nc.sync if half == 0 else nc.scalar
                eng.dma_start(
                    out=stage[:, half * 2048 : (half + 1) * 2048].rearrange(
                        "p (kc c) -> p kc c", kc=KC // 2
                    ),
                    in_=src_d[
                        half * 512 : 1024 if half else 512,
                        rb * 512 : (rb + 1) * 512,
                    ].rearrange("(kc p) c -> p kc c", p=128),
                )
            if kind in ("q", "k"):
                dst, b_sb = (qT, bq_sb) if kind == "q" else (kT, bk_sb)
                for mo in range(2):
                    pp_t = ps.tile([128, 512], F32, name="ps_proj", tag="ps_proj", bufs=2)
                    for kc in range(KC):
                        nc.tensor.matmul(
                            pp_t[:],
                            w_sb[:, kc * CH + mo * 128 : kc * CH + (mo + 1) * 128],
                            stage[:, kc * 512 : (kc + 1) * 512],
                            start=(kc == 0),
                            stop=(kc == KC - 1),
                        )
                    nc.vector.tensor_scalar(
                        out=dst[mo][:, rb * 512 : (rb + 1) * 512],
                        in0=pp_t[:],
                        scalar1=b_sb[:, mo : mo + 1],
                        scalar2=None,
                        op0=OP.add,
                    )
            else:
                for rs in range(4):
                    rc = rb * 4 + rs
                    pv_t = ps.tile([128, CH], F32, name="ps_v", tag="ps_proj", bufs=2)
                    for kc in range(KC):
                        nc.tensor.matmul(
                            pv_t[:],
                            stage[:, kc * 512 + rs * 128 : kc * 512 + (rs + 1) * 128],
                            wv_sb[:, kc * CH : (kc + 1) * CH],
                            start=(kc == 0),
                            stop=(kc == KC - 1),
                        )
                    # (v + bias) packed into the strided 65-wide layout, then
                    # scaled by the per-row mask (column rc = pp*KC+kc index)
                    vtmp = sb.tile([128, CH], F32, name="vtmp", tag="vtmp", bufs=2)
                    nc.vector.tensor_tensor(
                        out=vtmp[:], in0=pv_t[:], in1=bv_bc[:], op=OP.add
                    )
                    dst_ap = v_aug[
                        :, rc * NH * 65 : (rc + 1) * NH * 65
                    ].rearrange("p (h x) -> p h x", h=NH)[:, :, 0:DK]
                    nc.vector.tensor_scalar(
                        out=dst_ap,
                        in0=vtmp[:].rearrange("p (h x) -> p h x", h=NH),
                        scalar1=maskf[:, rc : rc + 1],
                        scalar2=None,
                        op0=OP.mult,
                    )
                    # mask the ones column too
                    nc.vector.tensor_scalar(
                        out=v_aug[
                            :, rc * NH * 65 : (rc + 1) * NH * 65
                        ].rearrange("p (h x) -> p h x", h=NH)[:, :, DK : DK + 1],
                        in0=maskf[:, rc : rc + 1]
                        .rearrange("p (a b) -> p a b", a=1)
                        .broadcast_to([128, NH, 1]),
                        scalar1=1.0,
                        scalar2=None,
                        op0=OP.mult,
                    )

    # ---- attention, per-head combine, interleaved output projection ---
    comb = [sb.tile([128, R], OUT_DT, name=f"comb{kk}") for kk in range(2)]

    def outproj_rc(p, rc, dma_eng):
        od = sb.tile([128, D], F32, name="od", tag="od", bufs=2)
        for ob in range(2):
            po_t = ps.tile([128, 512], F32, name="ps_o", tag="ps_proj", bufs=2)
            for kk in range(2):
                nc.tensor.matmul(
                    po_t[:],
                    comb[kk][:, p * S + rc * 128 : p * S + (rc + 1) * 128],
                    wo_sb[:, kk * D + ob * 512 : kk * D + (ob + 1) * 512],
                    start=(kk == 0),
                    stop=(kk == 1),
                )
            nc.vector.tensor_copy(od[:, ob * 512 : (ob + 1) * 512], po_t[:])
        dma_eng.dma_start(
            out=out_d[p * S + rc * 128 : p * S + (rc + 1) * 128, :], in_=od[:]
        )

    for p in range(P):
        for h in range(NH):
            avs = []
            for side in range(2):
                pp = p if side == 0 else 1 - p
                mo, po = h // 2, (h % 2) * 64
                ex = [
                    sb.tile([128, 4096], AV_DT, name="ex", tag="stage", bufs=3)
                    for _ in range(2)
                ]
                for kc in range(KC):
                    ss_t = ps.tile([128, 1024], F32, name="ps_s", tag="ps_s", bufs=3)
                    for qb in range(2):
                        nc.tensor.matmul(
                            ss_t[:, qb * 512 : (qb + 1) * 512],
                            kT[mo][po : po + 64, pp * S + kc * 128 : pp * S + (kc + 1) * 128],
                            qT[mo][po : po + 64, p * S + qb * 512 : p * S + (qb + 1) * 512],
                            start=True,
                            stop=True,
                        )
                    nc.scalar.activation(
                        ex[kc // 4][:, (kc % 4) * 1024 : (kc % 4 + 1) * 1024],
                        ss_t[:],
                        AF.Exp,
                        scale=0.125,
                    )
                av = sb.tile([65, S], F32, name="av", tag="avT", bufs=4)
                for qb in range(2):
                    pa_t = ps.tile([65, 512], F32, name="ps_av", tag="ps_proj", bufs=2)
                    for kc in range(KC):
                        nc.tensor.matmul(
                            pa_t[:],
                            v_aug[:, (pp * KC + kc) * NH * 65 + h * 65 : (pp * KC + kc) * NH * 65 + (h + 1) * 65],
                            ex[kc // 4][:, (kc % 4) * 1024 + qb * 512 : (kc % 4) * 1024 + (qb + 1) * 512],
                            start=(kc == 0),
                            stop=(kc == KC - 1),
                        )
                    nc.vector.tensor_copy(av[:, qb * 512 : (qb + 1) * 512], pa_t[:])
                avs.append(av)

            # ---- normalize + combine this head --------------------------
            avL, avR = avs
            srs = sb.tile([32, 64], F32, name="srs", tag="srs", bufs=2)
            nc.sync.dma_start(
                out=srs[0:16, :], in_=avL[64:65, :].rearrange("p (m e) -> p m e", e=64)
            )
            nc.sync.dma_start(
                out=srs[16:32, :], in_=avR[64:65, :].rearrange("p (m e) -> p m e", e=64)
            )
            rrs = sb.tile([32, 64], F32, name="rrs", tag="rrs", bufs=2)
            nc.vector.reciprocal(rrs[:], srs[:])
            rrL = sb.tile([1, S], F32, name="rrL", tag="rrow", bufs=2)
            rrR = sb.tile([1, S], F32, name="rrR", tag="rrow", bufs=2)
            nc.sync.dma_start(
                out=rrL[:].rearrange("p (m e) -> p m e", e=64), in_=rrs[0:16, :]
            )
            nc.sync.dma_start(
                out=rrR[:].rearrange("p (m e) -> p m e", e=64), in_=rrs[16:32, :]
            )
            po = (h % 2) * 64
            bcL = sb.tile([64, S], F32, name="bcL", tag="bc", bufs=2)
            bcR = sb.tile([64, S], F32, name="bcR", tag="bc", bufs=2)
            nc.gpsimd.partition_broadcast(bcL[:], rrL[:])
            nc.gpsimd.partition_broadcast(bcR[:], rrR[:])
            t1 = sb.tile([64, S], F32, name="t1", tag="t1", bufs=2)
            t2 = sb.tile([64, S], F32, name="t2", tag="t2", bufs=2)
            t3 = sb.tile([64, S], F32, name="t3", tag="t3", bufs=2)
            nc.vector.tensor_tensor(out=t1[:], in0=avL[0:64, :], in1=bcL[:], op=OP.mult)
            nc.vector.tensor_tensor(out=t2[:], in0=avR[0:64, :], in1=bcR[:], op=OP.mult)
            nc.scalar.activation(t3[:], t2[:], AF.Tanh)
            nc.vector.scalar_tensor_tensor(
                out=comb[h // 2][po : po + 64, p * S : (p + 1) * S],
                in0=t3[:],
                scalar=0.1,
                in1=t1[:],
                op0=OP.mult,
                op1=OP.add,
            )

            # interleave p0's output projection into p1's (ACT-bound)
            # attention stream so the PE never idles there
            if p == 1:
                rcs = [2 * h, 2 * h + 1]
                for rc in rcs:
                    outproj_rc(0, rc, nc.sync)

    for rc in range(8):
        outproj_rc(1, rc, nc.sync if rc % 2 == 0 else nc.scalar)


_CACHED = None


def _build():
    global _CACHED
    if _CACHED is not None:
        return _CACHED
    nc = bacc.Bacc("TRN2", target_bir_lowering=False, debug=False)
    xq = nc.dram_tensor("xq", [D, R], X_DT, kind="ExternalInput")
    xk = nc.dram_tensor("xk", [D, R], X_DT, kind="ExternalInput")
    xv = nc.dram_tensor("xv", [D, R], X_DT, kind="ExternalInput")
    wq = nc.dram_tensor("wq", [D, CH], X_DT, kind="ExternalInput")
    wk = nc.dram_tensor("wk", [D, CH], X_DT, kind="ExternalInput")
    wv = nc.dram_tensor("wv", [D, CH], X_DT, kind="ExternalInput")
    wo = nc.dram_tensor("wo", [CH, D], OUT_DT, kind="ExternalInput")
    bq = nc.dram_tensor("bq", [CH], F32, kind="ExternalInput")
    bk = nc.dram_tensor("bk", [CH], F32, kind="ExternalInput")
    bv = nc.dram_tensor("bv", [CH], F32, kind="ExternalInput")
    mask = nc.dram_tensor("mask", [P, S], I32, kind="ExternalInput")
    out_d = nc.dram_tensor("out", [R, D], F32, kind="ExternalOutput")
    with tile.TileContext(nc) as tc:
        _emit(nc, tc, xq, xk, xv, wq, wk, wv, wo, bq, bk, bv, mask, out_d)
    nc.compile()
    _CACHED = nc
    return nc


def _in_maps(query, key, value, mask, Wq, bq, Wk, bk, Wv, bv, Wo):
    xnp = mybir.dt.np(X_DT)
    onp = mybir.dt.np(OUT_DT)
    f32 = lambda a: np.ascontiguousarray(np.asarray(a, dtype=np.float32))
    xdt = lambda a: np.ascontiguousarray(np.asarray(a).astype(xnp))
    odt = lambda a: np.ascontiguousarray(np.asarray(a).astype(onp))
    query, key, value = f32(query), f32(key), f32(value)
    Wq, Wk, Wv, Wo = f32(Wq), f32(Wk), f32(Wv), f32(Wo)
    bq, bk, bv = f32(bq), f32(bk), f32(bv)
    mask = np.ascontiguousarray(np.asarray(mask, dtype=np.int32))

    xqT = [xdt(query[b].reshape(R, D).T) for b in range(B)]
    xkT = [xdt(key[b].reshape(R, D).T) for b in range(B)]
    xvT = [xdt(value[b].reshape(R, D).T) for b in range(B)]

    maps = []
    for c in range(NCORES):
        b, hg = divmod(c, GROUP)
        ch = slice(hg * CH, (hg + 1) * CH)
        maps.append(
            {
                "xq": xqT[b],
                "xk": xkT[b],
                "xv": xvT[b],
                "wq": xdt(Wq[ch, :].T),
                "wk": xdt(Wk[ch, :].T),
                "wv": xdt(Wv[ch, :].T),
                "wo": odt(Wo[:, ch].T),
                "bq": bq[ch],
                "bk": bk[ch],
                "bv": bv[ch],
                "mask": mask[b, :, 0, :],
            }
        )
    return maps


def _run(in_maps, **kwargs):
    nc = _build()
    return run_bass_kernel_spmd(nc, in_maps, core_ids=list(range(NCORES)), **kwargs)


def kernel(query, key, value, mask, Wq, bq, Wk, bk, Wv, bv, Wo, bo):
    res = _run(_in_maps(query, key, value, mask, Wq, bq, Wk, bk, Wv, bv, Wo))
    bo = np.asarray(bo, dtype=np.float32)
    out = np.zeros((B, P, S, D), dtype=np.float32)
    for c in range(NCORES):
        b = c // GROUP
        out[b] += res.results[c]["out"].reshape(P, S, D)
    out += bo
    return out


# revision 14
# speedup vs baseline: 1.1575x; 1.1231x over previous
"""Trainium2 Bass kernel for the paired-view ("flip") multi-head attention module.

Full computation (reference semantics, B=2 P=2 S=1024 D=1024 H=16):
    q/k/v = Linear(x) -> [B,P,H,S,DK]
    left  = softmax(q k^T / 8 + mask) v          (same pair index)
    right = softmax(q k_flip^T / 8 + mask) v_flip (pair index swapped)
    out   = (left + 0.1*tanh(right)) @ Wo.T + bo

Sharding over 8 NeuronCores: data-parallel on B (2 groups of 4 cores),
tensor-parallel on heads within a group (4 heads/core, 256 channels).
Each core computes its heads' projections (column-parallel), full attention
for its heads over both pair views, and a row-parallel partial of the output
projection.  The host sums the 4 partials per batch and adds bo.

Key layout trick: scores are computed TRANSPOSED ([k, q] instead of [q, k])
so softmax's exp is orientation-free and the attention-value product needs
no on-chip transposes; row sums come free via an extra ones-column in V.
All matmuls run in float32r (TF32-like, full PE rate at N>=512).
"""

import numpy as np

import concourse.bass as bass
import concourse.tile as tile
from concourse import bacc, mybir
from concourse.bass_utils import run_bass_kernel_spmd

F32 = mybir.dt.float32
F32R = mybir.dt.float32r
BF16 = mybir.dt.bfloat16
I32 = mybir.dt.int32

# per-stage matmul input dtypes (both operands of a matmul must match)
X_DT = BF16    # projection inputs: xT staging + Wq/Wk/Wv
QK_DT = BF16   # q/k tiles feeding the scores matmul
AV_DT = BF16   # exp(scores) + v_aug feeding the AV matmul
OUT_DT = BF16  # combine + Wo feeding the output projection
AF = mybir.ActivationFunctionType
OP = mybir.AluOpType

B, P, S, D, H = 2, 2, 1024, 1024, 16
DK = D // H          # 64
NCORES = 8
GROUP = 4            # cores per batch entry
NH = H // GROUP      # 4 local heads per core
CH = NH * DK         # 256 local channels
R = P * S            # 2048 rows per batch entry
KC = 8               # d_model chunks of 128
RB = 4               # row banks of 512
MASK_NEG = 60.0      # exp(-60) == 0 relative to any sum


def _emit(nc, tc, xq, xk, xv, wq, wk, wv, wo, bq, bk, bv, mask, out_d):
    from contextlib import ExitStack

    with ExitStack() as ctx:
        sb = ctx.enter_context(tc.tile_pool(name="sb", bufs=1))
        ps = ctx.enter_context(tc.tile_pool(name="ps", bufs=1, space="PSUM"))
        _body(nc, sb, ps, xq, xk, xv, wq, wk, wv, wo, bq, bk, bv, mask, out_d)


def _body(nc, sb, ps, xq, xk, xv, wq, wk, wv, wo, bq, bk, bv, mask, out_d):
    # ---- constants ----------------------------------------------------
    wq_sb = sb.tile([128, KC * CH], X_DT, name="wq_sb")
    wk_sb = sb.tile([128, KC * CH], X_DT, name="wk_sb")
    wv_sb = sb.tile([128, KC * CH], X_DT, name="wv_sb")
    for t_d, t_s in ((wq, wq_sb), (wk, wk_sb), (wv, wv_sb)):
        nc.gpsimd.dma_start(
            out=t_s[:].rearrange("p (kc c) -> p kc c", kc=KC),
            in_=t_d[:].rearrange("(kc p) c -> p kc c", p=128),
        )
    wo_sb = sb.tile([128, 2 * D], OUT_DT, name="wo_sb")
    nc.gpsimd.dma_start(
        out=wo_sb[:].rearrange("p (kk c) -> p kk c", kk=2),
        in_=wo[:].rearrange("(kk p) c -> p kk c", p=128),
    )

    bq_sb = sb.tile([128, 2], F32, name="bq_sb")
    bk_sb = sb.tile([128, 2], F32, name="bk_sb")
    nc.sync.dma_start(out=bq_sb[:], in_=bq[:].rearrange("(mo p) -> p mo", p=128))
    nc.sync.dma_start(out=bk_sb[:], in_=bk[:].rearrange("(mo p) -> p mo", p=128))
    bv_row = sb.tile([1, CH], F32, name="bv_row")
    nc.sync.dma_start(out=bv_row[:], in_=bv[None, :])
    bv_bc = sb.tile([128, CH], F32, name="bv_bc")
    nc.gpsimd.partition_broadcast(bv_bc[:], bv_row[:])

    # mask as a per-row 0/1 multiplier on v_aug (kills masked keys in both
    # the attention numerator and the ones-column denominator)
    mask_sb = sb.tile([128, 2 * KC], I32, name="mask_sb")
    nc.sync.dma_start(
        out=mask_sb[:],
        in_=mask[:].rearrange("pp (kc p) -> p pp kc", p=128),
    )
    maskf = sb.tile([128, 2 * KC], F32, name="maskf")
    nc.vector.tensor_scalar(
        out=maskf[:], in0=mask_sb[:], scalar1=1.0, scalar2=None, op0=OP.mult,
    )

    # ---- projections --------------------------------------------------
    # qT/kT: [o_local, p*S + s] in 2 tiles of 128 channels (2 heads each)
    qT = [sb.tile([128, R], QK_DT, name=f"qT{mo}") for mo in range(2)]
    kT = [sb.tile([128, R], QK_DT, name=f"kT{mo}") for mo in range(2)]
    # v_aug: [r_local, rc(16) x (h(4) x 65)]; col h*65+64 holds ones
    v_aug = sb.tile([128, 16 * NH * 65], AV_DT, name="v_aug")
    nc.gpsimd.memset(v_aug[:], 1.0)

    def proj_chunk(kind, rb):
        src_d = {"q": xq, "k": xk, "v": xv}[kind]
        w_sb = {"q": wq_sb, "k": wk_sb, "v": wv_sb}[kind]
        stage = sb.tile([128, KC * 512], X_DT, name="stage", tag="stage", bufs=2)
        for half in range(2):
            eng = nc.sync if half == 0 else nc.gpsimd
            eng.dma_start(
                out=stage[:, half * 2048 : (half + 1) * 2048].rearrange(
                    "p (kc c) -> p kc c", kc=KC // 2
                ),
                in_=src_d[
                    half * 512 : 1024 if half else 512,
                    rb * 512 : (rb + 1) * 512,
                ].rearrange("(kc p) c -> p kc c", p=128),
            )
        if kind in ("q", "k"):
            dst, b_sb = (qT, bq_sb) if kind == "q" else (kT, bk_sb)
            for mo in range(2):
                pp_t = ps.tile([128, 512], F32, name="ps_proj", tag="ps_proj", bufs=2)
                for kc in range(KC):
                    nc.tensor.matmul(
                        pp_t[:],
                        w_sb[:, kc * CH + mo * 128 : kc * CH + (mo + 1) * 128],
                        stage[:, kc * 512 : (kc + 1) * 512],
                        start=(kc == 0),
                        stop=(kc == KC - 1),
                    )
                nc.vector.tensor_scalar(
                    out=dst[mo][:, rb * 512 : (rb + 1) * 512],
                    in0=pp_t[:],
                    scalar1=b_sb[:, mo : mo + 1],
                    scalar2=None,
                    op0=OP.add,
                )
        else:
            for rs in range(4):
                rc = rb * 4 + rs
                pv_t = ps.tile([128, CH], F32, name="ps_v", tag="ps_proj", bufs=2)
                for kc in range(KC):
                    nc.tensor.matmul(
                        pv_t[:],
                        stage[:, kc * 512 + rs * 128 : kc * 512 + (rs + 1) * 128],
                        wv_sb[:, kc * CH : (kc + 1) * CH],
                        start=(kc == 0),
                        stop=(kc == KC - 1),
                    )
                vtmp = sb.tile([128, CH], F32, name="vtmp", tag="vtmp", bufs=2)
                nc.vector.tensor_tensor(
                    out=vtmp[:], in0=pv_t[:], in1=bv_bc[:], op=OP.add
                )
                dst_ap = v_aug[
                    :, rc * NH * 65 : (rc + 1) * NH * 65
                ].rearrange("p (h x) -> p h x", h=NH)[:, :, 0:DK]
                nc.vector.tensor_scalar(
                    out=dst_ap,
                    in0=vtmp[:].rearrange("p (h x) -> p h x", h=NH),
                    scalar1=maskf[:, rc : rc + 1],
                    scalar2=None,
                    op0=OP.mult,
                )
                nc.vector.tensor_scalar(
                    out=v_aug[
                        :, rc * NH * 65 : (rc + 1) * NH * 65
                    ].rearrange("p (h x) -> p h x", h=NH)[:, :, DK : DK + 1],
                    in0=maskf[:, rc : rc + 1]
                    .rearrange("p (a b) -> p a b", a=1)
                    .broadcast_to([128, NH, 1]),
                    scalar1=1.0,
                    scalar2=None,
                    op0=OP.mult,
                )

    # ---- attention building blocks ------------------------------------
    comb = [sb.tile([128, R], OUT_DT, name=f"comb{kk}") for kk in range(2)]

    def combo(p, h, side):
        pp = p if side == 0 else 1 - p
        mo, po = h // 2, (h % 2) * 64
        ex = [
            sb.tile([128, 4096], AV_DT, name="ex", tag="ex", bufs=3)
            for _ in range(2)
        ]
        for kc in range(KC):
            ss_t = ps.tile([128, 1024], F32, name="ps_s", tag="ps_s", bufs=3)
            for qb in range(2):
                nc.tensor.matmul(
                    ss_t[:, qb * 512 : (qb + 1) * 512],
                    kT[mo][po : po + 64, pp * S + kc * 128 : pp * S + (kc + 1) * 128],
                    qT[mo][po : po + 64, p * S + qb * 512 : p * S + (qb + 1) * 512],
                    start=True,
                    stop=True,
                )
            nc.scalar.activation(
                ex[kc // 4][:, (kc % 4) * 1024 : (kc % 4 + 1) * 1024],
                ss_t[:],
                AF.Exp,
                scale=0.125,
            )
        av = sb.tile([65, S], F32, name="av", tag="avT", bufs=6)
        for qb in range(2):
            pa_t = ps.tile([65, 512], F32, name="ps_av", tag="ps_proj", bufs=2)
            for kc in range(KC):
                nc.tensor.matmul(
                    pa_t[:],
                    v_aug[:, (pp * KC + kc) * NH * 65 + h * 65 : (pp * KC + kc) * NH * 65 + (h + 1) * 65],
                    ex[kc // 4][:, (kc % 4) * 1024 + qb * 512 : (kc % 4) * 1024 + (qb + 1) * 512],
                    start=(kc == 0),
                    stop=(kc == KC - 1),
                )
            nc.vector.tensor_copy(av[:, qb * 512 : (qb + 1) * 512], pa_t[:])
        return av

    def combine(p, h, avL, avR):
        srs = sb.tile([32, 64], F32, name="srs", tag="srs", bufs=2)
        nc.sync.dma_start(
            out=srs[0:16, :], in_=avL[64:65, :].rearrange("p (m e) -> p m e", e=64)
        )
        nc.sync.dma_start(
            out=srs[16:32, :], in_=avR[64:65, :].rearrange("p (m e) -> p m e", e=64)
        )
        rrs = sb.tile([32, 64], F32, name="rrs", tag="rrs", bufs=2)
        nc.vector.reciprocal(rrs[:], srs[:])
        rrL = sb.tile([1, S], F32, name="rrL", tag="rrow", bufs=2)
        rrR = sb.tile([1, S], F32, name="rrR", tag="rrow", bufs=2)
        nc.sync.dma_start(
            out=rrL[:].rearrange("p (m e) -> p m e", e=64), in_=rrs[0:16, :]
        )
        nc.sync.dma_start(
            out=rrR[:].rearrange("p (m e) -> p m e", e=64), in_=rrs[16:32, :]
        )
        po = (h % 2) * 64
        bcL = sb.tile([64, S], F32, name="bcL", tag="bc", bufs=2)
        bcR = sb.tile([64, S], F32, name="bcR", tag="bc", bufs=2)
        nc.gpsimd.partition_broadcast(bcL[:], rrL[:])
        nc.gpsimd.partition_broadcast(bcR[:], rrR[:])
        t1 = sb.tile([64, S], F32, name="t1", tag="t1", bufs=2)
        t2 = sb.tile([64, S], F32, name="t2", tag="t2", bufs=2)
        t3 = sb.tile([64, S], F32, name="t3", tag="t3", bufs=2)
        nc.vector.tensor_tensor(out=t1[:], in0=avL[0:64, :], in1=bcL[:], op=OP.mult)
        nc.vector.tensor_tensor(out=t2[:], in0=avR[0:64, :], in1=bcR[:], op=OP.mult)
        nc.scalar.activation(t3[:], t2[:], AF.Tanh)
        nc.vector.scalar_tensor_tensor(
            out=comb[h // 2][po : po + 64, p * S : (p + 1) * S],
            in0=t3[:],
            scalar=0.1,
            in1=t1[:],
            op0=OP.mult,
            op1=OP.add,
        )

    def outproj_rc(p, rc):
        od = sb.tile([128, D], F32, name="od", tag="od", bufs=2)
        for ob in range(2):
            po_t = ps.tile([128, 512], F32, name="ps_o", tag="ps_proj", bufs=2)
            for kk in range(2):
                nc.tensor.matmul(
                    po_t[:],
                    comb[kk][:, p * S + rc * 128 : p * S + (rc + 1) * 128],
                    wo_sb[:, kk * D + ob * 512 : kk * D + (ob + 1) * 512],
                    start=(kk == 0),
                    stop=(kk == 1),
                )
            nc.vector.tensor_copy(od[:, ob * 512 : (ob + 1) * 512], po_t[:])
        nc.sync.dma_start(
            out=out_d[p * S + rc * 128 : p * S + (rc + 1) * 128, :], in_=od[:]
        )

    # ---- schedule -----------------------------------------------------
    # lead-in: the p0 halves of k/v/q projections
    for kind, rb in (("k", 0), ("k", 1), ("v", 0), ("v", 1), ("q", 0), ("q", 1)):
        proj_chunk(kind, rb)

    # p0 side-0 attention, remaining projections interleaved into the
    # ACT-bound stream
    rest = [("k", 2), ("k", 3), ("v", 2), ("v", 3), ("q", 2), ("q", 3)]
    av0 = {}
    for h in range(NH):
        av0[h] = combo(0, h, 0)
        for _ in range(2 if h < 2 else 1):
            if rest:
                proj_chunk(*rest.pop(0))

    # p0 side-1 + per-head combine
    for h in range(NH):
        avR = combo(0, h, 1)
        combine(0, h, av0[h], avR)

    # p1 attention with p0's output projection interleaved
    for h in range(NH):
        avL = combo(1, h, 0)
        avR = combo(1, h, 1)
        combine(1, h, avL, avR)
        outproj_rc(0, 2 * h)
        outproj_rc(0, 2 * h + 1)

    for rc in range(8):
        outproj_rc(1, rc)


_CACHED = None


def _build():
    global _CACHED
    if _CACHED is not None:
        return _CACHED
    nc = bacc.Bacc("TRN2", target_bir_lowering=False, debug=False)
    xq = nc.dram_tensor("xq", [D, R], X_DT, kind="ExternalInput")
    xk = nc.dram_tensor("xk", [D, R], X_DT, kind="ExternalInput")
    xv = nc.dram_tensor("xv", [D, R], X_DT, kind="ExternalInput")
    wq = nc.dram_tensor("wq", [D, CH], X_DT, kind="ExternalInput")
    wk = nc.dram_tensor("wk", [D, CH], X_DT, kind="ExternalInput")
    wv = nc.dram_tensor("wv", [D, CH], X_DT, kind="ExternalInput")
    wo = nc.dram_tensor("wo", [CH, D], OUT_DT, kind="ExternalInput")
    bq = nc.dram_tensor("bq", [CH], F32, kind="ExternalInput")
    bk = nc.dram_tensor("bk", [CH], F32, kind="ExternalInput")
    bv = nc.dram_tensor("bv", [CH], F32, kind="ExternalInput")
    mask = nc.dram_tensor("mask", [P, S], I32, kind="ExternalInput")
    out_d = nc.dram_tensor("out", [R, D], F32, kind="ExternalOutput")
    with tile.TileContext(nc) as tc:
        _emit(nc, tc, xq, xk, xv, wq, wk, wv, wo, bq, bk, bv, mask, out_d)
    nc.compile()
    _CACHED = nc
    return nc


def _in_maps(query, key, value, mask, Wq, bq, Wk, bk, Wv, bv, Wo):
    xnp = mybir.dt.np(X_DT)
    onp = mybir.dt.np(OUT_DT)
    f32 = lambda a: np.ascontiguousarray(np.asarray(a, dtype=np.float32))
    xdt = lambda a: np.ascontiguousarray(np.asarray(a).astype(xnp))
    odt = lambda a: np.ascontiguousarray(np.asarray(a).astype(onp))
    query, key, value = f32(query), f32(key), f32(value)
    Wq, Wk, Wv, Wo = f32(Wq), f32(Wk), f32(Wv), f32(Wo)
    bq, bk, bv = f32(bq), f32(bk), f32(bv)
    mask = np.ascontiguousarray(np.asarray(mask, dtype=np.int32))

    xqT = [xdt(query[b].reshape(R, D).T) for b in range(B)]
    xkT = [xdt(key[b].reshape(R, D).T) for b in range(B)]
    xvT = [xdt(value[b].reshape(R, D).T) for b in range(B)]

    maps = []
    for c in range(NCORES):
        b, hg = divmod(c, GROUP)
        ch = slice(hg * CH, (hg + 1) * CH)
        maps.append(
            {
                "xq": xqT[b],
                "xk": xkT[b],
                "xv": xvT[b],
                "wq": xdt(Wq[ch, :].T),
                "wk": xdt(Wk[ch, :].T),
                "wv": xdt(Wv[ch, :].T),
                "wo": odt(Wo[:, ch].T),
                "bq": bq[ch],
                "bk": bk[ch],
                "bv": bv[ch],
                "mask": mask[b, :, 0, :],
            }
        )
    return maps


def _run(in_maps, **kwargs):
    nc = _build()
    return run_bass_kernel_spmd(nc, in_maps, core_ids=list(range(NCORES)), **kwargs)


def kernel(query, key, value, mask, Wq, bq, Wk, bk, Wv, bv, Wo, bo):
    res = _run(_in_maps(query, key, value, mask, Wq, bq, Wk, bk, Wv, bv, Wo))
    bo = np.asarray(bo, dtype=np.float32)
    out = np.zeros((B, P, S, D), dtype=np.float32)
    for c in range(NCORES):
        b = c // GROUP
        out[b] += res.results[c]["out"].reshape(P, S, D)
    out += bo
    return out
